# revision 6
# baseline (speedup 1.0000x reference)
"""3-layer GCN (GCNConv x3 + LeakyReLU, PyG semantics) on 8 Trainium2 cores.

Strategy (graph-parallel over destination nodes):
  - Nodes are partitioned into 8 contiguous ranges; core c owns range c and
    computes the output rows for its own nodes.
  - Table rows are laid out PERMUTED: row'(v) = c*RPC + r*TILES + tc where
    v = c*NPC + tc*128 + r.  A phase-A chunk [128, TILES, 2D] then writes to
    DRAM with per-partition-contiguous runs (fast big-elem DMA), and bucket
    b = rows [b*25088, (b+1)*25088) corresponds to cores {2b, 2b+1}.
  - Layer 1: x is a full input, so EVERY core computes the full table
    locally (replicated phase A from xT in bf16) -- no AllGather at all.
  - Layers 2-3: phase A computes the own chunk; AllGather builds the full
    table.
  - Self-loops are appended as ordinary edges; the bias is injected via a
    rank-1 matmul (sqrt(deg) outer b) into the same PSUM accumulator, so the
    epilogue is a single Activation-engine LeakyRelu with per-partition
    dinv scale:  h = lrelu(dinv * (sum_edges table[src] + sqrtdeg*b)).
  - dma_gather indices are int16; the table (100352 rows) is addressed
    through 4 bucket views of 25088 rows; edges are bucketed by row' range.

The Bass program is SPMD: one program, per-core input data. Per
(tile, bucket) section lengths are shared across cores (max over cores,
padded with dummy edges whose one-hot column is zero: dstrel = -1).
"""
import sys

sys.path.insert(0, "/opt/trn_rl_repo")

import numpy as np

import concourse.bacc as bacc
import concourse.mybir as mybir
import concourse.tile as tile
from concourse import library_config
from concourse.bass_utils import run_bass_kernel_spmd
from concourse.masks import make_identity

_F32 = mybir.dt.float32
_BF16 = mybir.dt.bfloat16
_I16 = mybir.dt.int16
P = 128
D = 64
NEG_SLOPE = 0.01
ACT = mybir.ActivationFunctionType


class Cfg:
    def __init__(self, n_nodes=100000, cores=8, group=4, layers=3):
        self.N = n_nodes
        self.CORES = cores
        self.NPC = self.N // cores            # nodes owned per core
        self.TILES = (self.NPC + P - 1) // P  # dst tiles per core
        self.RPC = self.TILES * P             # padded rows per core
        self.GR = cores * self.RPC            # gather-table rows
        self.NBUCK = max(1, -(-self.GR // 25088))
        assert self.GR % self.NBUCK == 0
        self.BUCK_ROWS = self.GR // self.NBUCK
        assert self.BUCK_ROWS <= 32767
        self.GROUP = group                    # dst tiles per gather group
        self.LAYERS = layers
        self.ABLATE = set()
        self.GCAP = 32                        # max 128-blocks per dma_gather
        self.SPACK = False
        self.MSGBUFS = 2
        self.QBUFS = 4
        self.PSABUFS = 2
        self.WKBUFS = 3


DEFAULT_CFG = Cfg()


def _preprocess(edge_index, cfg):
    """Sort/bucket/pack edges; build per-core device arrays and metadata."""
    N, CORES, NPC, RPC = cfg.N, cfg.CORES, cfg.NPC, cfg.RPC
    TILES, NBUCK, BUCK_ROWS = cfg.TILES, cfg.NBUCK, cfg.BUCK_ROWS

    rsrc = np.asarray(edge_index[0], dtype=np.int64)
    rdst = np.asarray(edge_index[1], dtype=np.int64)

    deg = np.bincount(rdst, minlength=N).astype(np.float32) + 1.0  # + self loop
    dinv = (1.0 / np.sqrt(deg)).astype(np.float32)

    # append self loops as ordinary edges
    loops = np.arange(N, dtype=np.int64)
    src = np.concatenate([rsrc, loops])
    dst = np.concatenate([rdst, loops])

    c_s = src // NPC
    loc_s = src - c_s * NPC
    rowp = c_s * RPC + (loc_s % P) * TILES + (loc_s // P)  # permuted table row
    bucket = rowp // BUCK_ROWS
    lidx = rowp % BUCK_ROWS

    owner = dst // NPC
    dloc = dst - owner * NPC
    tile_id = dloc // P
    dstrel = dloc % P

    counts = np.zeros((CORES, TILES, NBUCK), dtype=np.int64)
    np.add.at(counts, (owner, tile_id, bucket), 1)
    order = np.lexsort((bucket, tile_id, owner))
    sl = lidx[order]
    sr = dstrel[order]

    sec_len = counts.max(axis=0)              # [TILES, NBUCK] shared sections
    groups = [list(range(g, min(g + cfg.GROUP, TILES)))
              for g in range(0, TILES, cfg.GROUP)]

    # layout: group -> bucket -> tile sections back-to-back, span padded to
    # a multiple of 128 at the end
    ginfos = []
    tot_blocks = 0
    tot_cols = 0
    for grp in groups:
        gi = {"tiles": list(grp), "blk0": tot_blocks, "col0": tot_cols,
              "spans": {}, "tb": {}}
        gblk = 0
        gcol = 0
        for b in range(NBUCK):
            span_len = int(sec_len[grp, b].sum())
            if span_len == 0:
                continue
            kgb = -(-span_len // P)
            gi["spans"][b] = (gblk, kgb)
            so = 0
            for t in grp:
                stb = int(sec_len[t, b])
                if stb == 0:
                    continue
                j0 = so // P
                njt = (so + stb - 1) // P - j0 + 1
                gi["tb"][(t, b)] = (gblk + j0, njt, gcol, so)
                gcol += njt
                so += stb
            gblk += kgb
        gi["blocks"] = gblk
        gi["ncols"] = gcol
        tot_blocks += gblk
        tot_cols += gcol
        ginfos.append(gi)
    tot_idx = tot_blocks * P

    # boundaries of each core's (t, b) run in the sorted edge list
    cum = np.zeros((CORES, TILES, NBUCK + 1), dtype=np.int64)
    cum[:, :, 1:] = np.cumsum(counts, axis=2)
    flat_counts = counts.sum(axis=2)
    run = np.cumsum(flat_counts.reshape(-1))
    base = np.zeros(CORES * TILES, dtype=np.int64)
    base[1:] = run[:-1]
    base = base.reshape(CORES, TILES)

    per_core = []
    for c in range(CORES):
        lidx_flat = np.zeros(tot_idx, dtype=np.int16)
        drel = np.full((P, tot_cols), -1.0, dtype=np.float32)
        for gi in ginfos:
            for b, (bo, kgb) in gi["spans"].items():
                span_i0 = (gi["blk0"] + bo) * P
                for t in gi["tiles"]:
                    if (t, b) not in gi["tb"]:
                        continue
                    jb, njt, colg, so = gi["tb"][(t, b)]
                    n = int(counts[c, t, b])
                    if n == 0:
                        continue
                    s0 = int(base[c, t] + cum[c, t, b])
                    pos0 = span_i0 + so
                    lidx_flat[pos0:pos0 + n] = sl[s0:s0 + n].astype(np.int16)
                    q = so + np.arange(n)
                    jrel = q // P - so // P
                    pp = (pos0 + np.arange(n)) % P
                    cols = gi["col0"] + colg + jrel
                    drel[pp, cols] = sr[s0:s0 + n].astype(np.float32)
        idx16 = np.tile(lidx_flat.reshape(tot_idx // 16, 16).T, (8, 1)).copy()
        per_core.append({"idx16": idx16, "dstrel": drel})

    meta = {
        "sec_len": sec_len,
        "groups": groups,
        "ginfos": ginfos,
        "tot_idx": tot_idx,
        "tot_cols": tot_cols,
        "dinv": dinv,
    }
    return meta, per_core


def _build_program(meta, cfg):
    ginfos = meta["ginfos"]
    tot_idx = meta["tot_idx"]
    tot_cols = meta["tot_cols"]
    CORES, TILES, RPC, GR = cfg.CORES, cfg.TILES, cfg.RPC, cfg.GR
    NBUCK, BUCK_ROWS = cfg.NBUCK, cfg.BUCK_ROWS

    gblk_max = max(gi["blocks"] for gi in ginfos)
    gcol_max = max(gi["ncols"] for gi in ginfos)
    kmaxb = max((tb[1] for gi in ginfos for tb in gi["tb"].values()),
                default=1)

    nc = bacc.Bacc("TRN2", debug=False)
    nc.num_devices = CORES

    xTf_in = nc.dram_tensor("xTf", [D, CORES * RPC], _BF16,
                            kind="ExternalInput")
    dinvf_in = nc.dram_tensor("dinvf", [P, CORES * TILES], _F32,
                              kind="ExternalInput")
    dinv1_in = nc.dram_tensor("dinv1", [P, TILES], _F32, kind="ExternalInput")
    sqd_in = nc.dram_tensor("sqd", [1, RPC], _F32, kind="ExternalInput")
    w_in = [nc.dram_tensor(f"W{i + 1}", [D, D], _BF16, kind="ExternalInput")
            for i in range(3)]
    bias_in = [nc.dram_tensor(f"bias{i + 1}", [1, D], _F32,
                              kind="ExternalInput") for i in range(3)]
    iota_in = nc.dram_tensor("iota", [P, P], _BF16, kind="ExternalInput")
    idx_in = nc.dram_tensor("idx16", [P, tot_idx // 16], _I16,
                            kind="ExternalInput")
    drel_in = nc.dram_tensor("dstrel", [P, tot_cols], _F32,
                             kind="ExternalInput")
    out_t = nc.dram_tensor("out", [RPC, D], _F32, kind="ExternalOutput")

    with tile.TileContext(nc) as tc:
        with tc.tile_pool(name="dram", bufs=1, space="DRAM") as dram, \
             tc.tile_pool(name="const", bufs=1) as cst, \
             tc.tile_pool(name="persist", bufs=1) as per, \
             tc.tile_pool(name="xp", bufs=2) as xp, \
             tc.tile_pool(name="chp", bufs=2) as chp, \
             tc.tile_pool(name="msgp", bufs=cfg.MSGBUFS) as msgp, \
             tc.tile_pool(name="idxp", bufs=2) as idxp, \
             tc.tile_pool(name="qp", bufs=cfg.QBUFS) as qp, \
             tc.tile_pool(name="wk", bufs=cfg.WKBUFS) as wk, \
             tc.tile_pool(name="psa", bufs=cfg.PSABUFS, space="PSUM") as psa, \
             tc.tile_pool(name="psg", bufs=2, space="PSUM") as psg, \
             tc.tile_pool(name="pst", bufs=2, space="PSUM") as pst:

            nc.gpsimd.load_library(library_config.mlp)

            g_fulls = [dram.tile([GR, 2 * D], _BF16,
                                 addr_space=("Local" if i == 0 else "Shared"),
                                 name=f"g_full{i}")
                       for i in range(cfg.LAYERS)]
            g_own = dram.tile([RPC, 2 * D], _BF16, name="g_own")

            iota = cst.tile([P, P], _BF16)
            nc.sync.dma_start(iota[:], iota_in[:])
            ident = cst.tile([P, P], _F32)
            make_identity(nc, ident[:])
            dinvf = cst.tile([P, CORES * TILES], _F32)
            nc.sync.dma_start(dinvf[:], dinvf_in[:])
            dinv1 = cst.tile([P, TILES], _F32)
            nc.sync.dma_start(dinv1[:], dinv1_in[:])
            sqd = cst.tile([1, RPC], _F32)
            nc.sync.dma_start(sqd[:], sqd_in[:])
            ws, bs = [], []
            for i in range(3):
                w = cst.tile([D, D], _BF16, name=f"w{i}")
                nc.sync.dma_start(w[:], w_in[i][:])
                ws.append(w)
                bt = cst.tile([1, D], _F32, name=f"b{i}")
                nc.sync.dma_start(bt[:], bias_in[i][:])
                bs.append(bt)

            ht = per.tile([D, RPC], _BF16)          # H.T own (layer input)

            for L in range(cfg.LAYERS):
                gf = g_fulls[L]
                # ---------- phase A: table = dinv * (H @ W) ----------
                HC = TILES // 2  # half-chunk tiles
                if L == 0:
                    # replicated: every core computes the FULL table from x
                    for c in range(CORES):
                        for hh in range(2):
                            t0 = hh * HC
                            xc = xp.tile([D, HC * P], _BF16, tag="xc",
                                         name=f"xc{c}_{hh}")
                            nc.sync.dma_start(
                                xc[:], xTf_in[:, c * RPC + t0 * P:
                                              c * RPC + (t0 + HC) * P])
                            gch = chp.tile([P, HC, 2 * D], _BF16, tag="gch",
                                           name=f"gch0_{c}_{hh}")
                            for i in range(HC):
                                t = t0 + i
                                pg = psg.tile([P, D], _F32, tag="pg",
                                              name=f"pg0_{c}_{t}")
                                nc.tensor.matmul(
                                    pg[:], lhsT=xc[:, i * P:(i + 1) * P],
                                    rhs=ws[0][:], start=True, stop=True)
                                gt = c * TILES + t
                                nc.scalar.activation(
                                    gch[:, i, :D], pg[:], ACT.Copy,
                                    scale=dinvf[:, gt:gt + 1])
                            nc.sync.dma_start(
                                gf[c * RPC:(c + 1) * RPC, :].rearrange(
                                    "(r t) d -> r t d", r=P, t=TILES
                                )[:, t0:t0 + HC, :],
                                gch[:])
                else:
                    for hh in range(2):
                        t0 = hh * HC
                        gch = chp.tile([P, HC, 2 * D], _BF16, tag="gch",
                                       name=f"gch{L}_{hh}")
                        for i in range(HC):
                            t = t0 + i
                            pg = psg.tile([P, D], _F32, tag="pg",
                                          name=f"pg{L}_{t}")
                            nc.tensor.matmul(
                                pg[:], lhsT=ht[:, t * P:(t + 1) * P],
                                rhs=ws[L][:], start=True, stop=True)
                            nc.scalar.activation(
                                gch[:, i, :D], pg[:], ACT.Copy,
                                scale=dinv1[:, t:t + 1])
                        nc.sync.dma_start(
                            g_own[:].rearrange(
                                "(r t) d -> r t d", r=P, t=TILES
                            )[:, t0:t0 + HC, :],
                            gch[:])
                    # ---------- phase B: AllGather ----------
                    if "ag" not in cfg.ABLATE:
                        nc.gpsimd.collective_compute(
                            "AllGather",
                            mybir.AluOpType.bypass,
                            replica_groups=[list(range(CORES))],
                            ins=[g_own[:]],
                            outs=[gf[:]],
                        )

                # ---------- phase C: edge aggregation ----------
                for gi_i, gi in enumerate(ginfos):
                    blocks = gi["blocks"]
                    i0 = gi["blk0"] * P
                    nidx_g = blocks * P
                    msg = msgp.tile([P, gblk_max, 2 * D], _BF16, tag="msg",
                                    name=f"msg{L}_{gi_i}",
                                    padded_shape=[P, gblk_max, 2 * D])
                    idx_sb = idxp.tile([P, (gblk_max * P) // 16], _I16,
                                       tag="idx", name=f"idx{L}_{gi_i}",
                                       padded_shape=[P, (gblk_max * P) // 16])
                    drel_sb = idxp.tile([P, gcol_max], _F32, tag="drel",
                                        name=f"drel{L}_{gi_i}",
                                        padded_shape=[P, gcol_max])
                    nc.sync.dma_start(
                        idx_sb[:, :nidx_g // 16],
                        idx_in[:, i0 // 16:(i0 + nidx_g) // 16],
                    )
                    nc.sync.dma_start(
                        drel_sb[:, :gi["ncols"]],
                        drel_in[:, gi["col0"]:gi["col0"] + gi["ncols"]],
                    )
                    for b, (bo, kgb) in gi["spans"].items():
                        if "gather" in cfg.ABLATE:
                            continue
                        for s0 in range(0, kgb, cfg.GCAP):
                            kk = min(cfg.GCAP, kgb - s0)
                            bo2 = bo + s0
                            nidx = kk * P
                            nc.gpsimd.dma_gather(
                                msg[:, bo2:bo2 + kk, :],
                                gf[b * BUCK_ROWS:(b + 1) * BUCK_ROWS, :],
                                idx_sb[:, bo2 * P // 16:
                                       (bo2 * P + nidx) // 16],
                                nidx, nidx, 2 * D,
                                single_packet=cfg.SPACK,
                            )

                    for t in gi["tiles"]:
                        tbs = [(b, gi["tb"][(t, b)]) for b in range(NBUCK)
                               if (t, b) in gi["tb"]]
                        if "mm" in cfg.ABLATE:
                            tbs = []
                        pa = psa.tile([P, D], _F32, tag="pa",
                                      name=f"pa{L}_{t}")
                        # bias via rank-1 matmul: pa += sqrtdeg^T b
                        nc.tensor.matmul(
                            pa[:], lhsT=sqd[0:1, t * P:(t + 1) * P],
                            rhs=bs[L][:], start=True, stop=(len(tbs) == 0))
                        for bi, (b, (jb, njt, colg, so)) in enumerate(tbs):
                            q = qp.tile([P, njt, P], _BF16, tag="q",
                                        name=f"q{L}_{t}_{b}",
                                        padded_shape=[P, kmaxb, P])
                            if "q" not in cfg.ABLATE:
                                for j in range(njt):
                                    nc.vector.tensor_scalar(
                                        out=q[:, j, :], in0=iota[:],
                                        scalar1=drel_sb[:, colg + j:
                                                        colg + j + 1],
                                        scalar2=None,
                                        op0=mybir.AluOpType.is_equal,
                                    )
                            for j in range(njt):
                                nc.tensor.matmul(
                                    pa[:], lhsT=q[:, j, :],
                                    rhs=msg[:, jb + j, :D],
                                    start=False,
                                    stop=(bi == len(tbs) - 1 and
                                          j == njt - 1),
                                )
                        h = wk.tile([P, D], _F32, tag="h", name=f"h{L}_{t}")
                        nc.scalar.activation(
                            h[:], pa[:], ACT.Lrelu,
                            scale=dinv1[:, t:t + 1], alpha=NEG_SLOPE)
                        if L < cfg.LAYERS - 1:
                            pt = pst.tile([D, P], _F32, tag="pt",
                                          name=f"pt{L}_{t}")
                            nc.tensor.transpose(pt[:], h[:], ident[:])
                            nc.scalar.activation(
                                ht[:, t * P:(t + 1) * P], pt[:], ACT.Copy)
                        else:
                            nc.sync.dma_start(out_t[t * P:(t + 1) * P, :],
                                              h[:])

    nc.compile()
    return nc


def make_in_maps(x, Ws, bss, meta, per_core, cfg):
    dinv = meta["dinv"]
    CORES, NPC, RPC, TILES = cfg.CORES, cfg.NPC, cfg.RPC, cfg.TILES
    import ml_dtypes
    iota_np = np.broadcast_to(np.arange(P).astype(ml_dtypes.bfloat16),
                              (P, P)).copy()

    # full x, transposed, tile-padded per core block: [D, CORES*RPC]
    xpad = np.zeros((CORES * RPC, D), np.float32)
    for c in range(CORES):
        xpad[c * RPC:c * RPC + NPC] = x[c * NPC:(c + 1) * NPC]
    xTf = xpad.T.astype(ml_dtypes.bfloat16).copy()

    # dinv in permuted layout [128, CORES*TILES]
    dpad = np.zeros((CORES * RPC,), np.float32)
    for c in range(CORES):
        dpad[c * RPC:c * RPC + NPC] = dinv[c * NPC:(c + 1) * NPC]
    dinvf = dpad.reshape(CORES * TILES, P).T.copy()  # [r, c*TILES+t]

    in_maps = []
    for c in range(CORES):
        d1 = dinvf[:, c * TILES:(c + 1) * TILES].copy()
        dq = dpad[c * RPC:(c + 1) * RPC]
        sq = np.zeros((1, RPC), np.float32)
        nz = dq > 0
        sq[0, nz] = 1.0 / dq[nz]
        im = {
            "xTf": xTf,
            "dinvf": dinvf,
            "dinv1": d1,
            "sqd": sq,
            "iota": iota_np,
            "idx16": per_core[c]["idx16"],
            "dstrel": per_core[c]["dstrel"],
        }
        for i in range(3):
            im[f"W{i + 1}"] = Ws[i].astype(ml_dtypes.bfloat16)
            im[f"bias{i + 1}"] = bss[i].reshape(1, D).astype(np.float32)
        in_maps.append(im)
    return in_maps


_CACHE = {}


def kernel(x, edge_index, W1, b1, W2, b2, W3, b3):
    cfg = DEFAULT_CFG
    x = np.asarray(x, dtype=np.float32)
    Ws = [np.asarray(w, dtype=np.float32) for w in (W1, W2, W3)]
    bss = [np.asarray(b, dtype=np.float32) for b in (b1, b2, b3)]

    ei = np.asarray(edge_index)
    key = hash(ei[:, ::997].tobytes()) ^ hash(ei.shape)
    if key not in _CACHE:
        meta, per_core = _preprocess(ei, cfg)
        nc = _build_program(meta, cfg)
        _CACHE[key] = (meta, per_core, nc)
    meta, per_core, nc = _CACHE[key]

    in_maps = make_in_maps(x, Ws, bss, meta, per_core, cfg)
    res = run_bass_kernel_spmd(nc, in_maps, core_ids=list(range(cfg.CORES)))
    out = np.empty((cfg.N, D), np.float32)
    for c in range(cfg.CORES):
        out[c * cfg.NPC:(c + 1) * cfg.NPC] = res.results[c]["out"][:cfg.NPC]
    return out


if __name__ == "__main__":
    # quick smoke: build only
    rng = np.random.default_rng(0)
    ei = rng.integers(0, DEFAULT_CFG.N, size=(2, 1600000))
    meta, per_core = _preprocess(ei, DEFAULT_CFG)
    print("tot_idx", meta["tot_idx"], "tot_cols", meta["tot_cols"])
    nc = _build_program(meta, DEFAULT_CFG)
    print("build ok")


# revision 24
# speedup vs baseline: 1.0583x; 1.0583x over previous
"""3-layer GCN (GCNConv x3 + LeakyReLU, PyG semantics) on 8 Trainium2 cores.

Strategy (graph-parallel over destination nodes):
  - Nodes are partitioned into 8 contiguous ranges; core c owns range c and
    computes the output rows for its own nodes.
  - Table rows are laid out PERMUTED: row'(v) = c*RPC + r*TILES + tc where
    v = c*NPC + tc*128 + r.  A phase-A chunk [128, TILES, 2D] then writes to
    DRAM with per-partition-contiguous runs (fast big-elem DMA), and bucket
    b = rows [b*25088, (b+1)*25088) corresponds to cores {2b, 2b+1}.
  - Layer 1: x is a full input, so EVERY core computes the full table
    locally (replicated phase A from xT in bf16) -- no AllGather at all.
  - Layers 2-3: phase A computes the own chunk; AllGather builds the full
    table.
  - Self-loops are appended as ordinary edges; the bias is injected via a
    rank-1 matmul (sqrt(deg) outer b) into the same PSUM accumulator, so the
    epilogue is a single Activation-engine LeakyRelu with per-partition
    dinv scale:  h = lrelu(dinv * (sum_edges table[src] + sqrtdeg*b)).
  - dma_gather indices are int16; the table (100352 rows) is addressed
    through 4 bucket views of 25088 rows; edges are bucketed by row' range.

The Bass program is SPMD: one program, per-core input data. Per
(tile, bucket) section lengths are shared across cores (max over cores,
padded with dummy edges whose one-hot column is zero: dstrel = -1).
"""
import sys

sys.path.insert(0, "/opt/trn_rl_repo")

import numpy as np

import concourse.bacc as bacc
import concourse.mybir as mybir
import concourse.tile as tile
from concourse import library_config
from concourse.bass_utils import run_bass_kernel_spmd
from concourse.masks import make_identity

_F32 = mybir.dt.float32
_BF16 = mybir.dt.bfloat16
_I16 = mybir.dt.int16
P = 128
D = 64
NEG_SLOPE = 0.01
ACT = mybir.ActivationFunctionType


class Cfg:
    def __init__(self, n_nodes=100000, cores=8, group=4, layers=3):
        self.N = n_nodes
        self.CORES = cores
        self.NPC = self.N // cores            # nodes owned per core
        self.TILES = (self.NPC + P - 1) // P  # dst tiles per core
        self.RPC = self.TILES * P             # padded rows per core
        self.GR = cores * self.RPC            # gather-table rows
        self.NBUCK = max(1, -(-self.GR // 25088))
        assert self.GR % self.NBUCK == 0
        self.BUCK_ROWS = self.GR // self.NBUCK
        assert self.BUCK_ROWS <= 32767
        self.GROUP = group                    # dst tiles per gather group
        self.LAYERS = layers
        self.ABLATE = set()
        self.GCAP = 32                        # max 128-blocks per dma_gather
        self.SPACK = False
        self.MSGBUFS = 2
        self.QBUFS = 6
        self.PSABUFS = 4
        self.WKBUFS = 3


DEFAULT_CFG = Cfg()


def _preprocess(edge_index, cfg):
    """Sort/bucket/pack edges; build per-core device arrays and metadata."""
    N, CORES, NPC, RPC = cfg.N, cfg.CORES, cfg.NPC, cfg.RPC
    TILES, NBUCK, BUCK_ROWS = cfg.TILES, cfg.NBUCK, cfg.BUCK_ROWS

    rsrc = np.asarray(edge_index[0], dtype=np.int64)
    rdst = np.asarray(edge_index[1], dtype=np.int64)

    deg = np.bincount(rdst, minlength=N).astype(np.float32) + 1.0  # + self loop
    dinv = (1.0 / np.sqrt(deg)).astype(np.float32)

    # self loops handled by an identity matmul against the phase-A chunk
    src = rsrc
    dst = rdst

    c_s = src // NPC
    loc_s = src - c_s * NPC
    rowp = c_s * RPC + (loc_s % P) * TILES + (loc_s // P)  # permuted table row
    bucket = rowp // BUCK_ROWS
    lidx = rowp % BUCK_ROWS

    owner = dst // NPC
    dloc = dst - owner * NPC
    tile_id = dloc // P
    dstrel = dloc % P

    counts = np.zeros((CORES, TILES, NBUCK), dtype=np.int64)
    np.add.at(counts, (owner, tile_id, bucket), 1)
    order = np.lexsort((bucket, tile_id, owner))
    sl = lidx[order]
    sr = dstrel[order]

    sec_len = counts.max(axis=0)              # [TILES, NBUCK] shared sections
    groups = [list(range(g, min(g + cfg.GROUP, TILES)))
              for g in range(0, TILES, cfg.GROUP)]

    # layout: group -> bucket -> tile sections back-to-back, span padded to
    # a multiple of 128 at the end
    ginfos = []
    tot_blocks = 0
    tot_cols = 0
    for grp in groups:
        gi = {"tiles": list(grp), "blk0": tot_blocks, "col0": tot_cols,
              "spans": {}, "tb": {}}
        gblk = 0
        gcol = 0
        for b in range(NBUCK):
            span_len = int(sec_len[grp, b].sum())
            if span_len == 0:
                continue
            kgb = -(-span_len // P)
            gi["spans"][b] = (gblk, kgb)
            so = 0
            for t in grp:
                stb = int(sec_len[t, b])
                if stb == 0:
                    continue
                j0 = so // P
                njt = (so + stb - 1) // P - j0 + 1
                gi["tb"][(t, b)] = (gblk + j0, njt, gcol, so)
                gcol += njt
                so += stb
            gblk += kgb
        gi["blocks"] = gblk
        gi["ncols"] = gcol
        tot_blocks += gblk
        tot_cols += gcol
        ginfos.append(gi)
    tot_idx = tot_blocks * P

    # boundaries of each core's (t, b) run in the sorted edge list
    cum = np.zeros((CORES, TILES, NBUCK + 1), dtype=np.int64)
    cum[:, :, 1:] = np.cumsum(counts, axis=2)
    flat_counts = counts.sum(axis=2)
    run = np.cumsum(flat_counts.reshape(-1))
    base = np.zeros(CORES * TILES, dtype=np.int64)
    base[1:] = run[:-1]
    base = base.reshape(CORES, TILES)

    per_core = []
    for c in range(CORES):
        lidx_flat = np.zeros(tot_idx, dtype=np.int16)
        drel = np.full((P, tot_cols), -1.0, dtype=np.float32)
        for gi in ginfos:
            for b, (bo, kgb) in gi["spans"].items():
                span_i0 = (gi["blk0"] + bo) * P
                for t in gi["tiles"]:
                    if (t, b) not in gi["tb"]:
                        continue
                    jb, njt, colg, so = gi["tb"][(t, b)]
                    n = int(counts[c, t, b])
                    if n == 0:
                        continue
                    s0 = int(base[c, t] + cum[c, t, b])
                    pos0 = span_i0 + so
                    lidx_flat[pos0:pos0 + n] = sl[s0:s0 + n].astype(np.int16)
                    q = so + np.arange(n)
                    jrel = q // P - so // P
                    pp = (pos0 + np.arange(n)) % P
                    cols = gi["col0"] + colg + jrel
                    drel[pp, cols] = sr[s0:s0 + n].astype(np.float32)
        idx16 = np.tile(lidx_flat.reshape(tot_idx // 16, 16).T, (8, 1)).copy()
        per_core.append({"idx16": idx16, "dstrel": drel})

    meta = {
        "sec_len": sec_len,
        "groups": groups,
        "ginfos": ginfos,
        "tot_idx": tot_idx,
        "tot_cols": tot_cols,
        "dinv": dinv,
    }
    return meta, per_core


def _build_program(meta, cfg):
    ginfos = meta["ginfos"]
    tot_idx = meta["tot_idx"]
    tot_cols = meta["tot_cols"]
    CORES, TILES, RPC, GR = cfg.CORES, cfg.TILES, cfg.RPC, cfg.GR
    NBUCK, BUCK_ROWS = cfg.NBUCK, cfg.BUCK_ROWS

    gblk_max = max(gi["blocks"] for gi in ginfos)
    gcol_max = max(gi["ncols"] for gi in ginfos)
    kmaxb = max((tb[1] for gi in ginfos for tb in gi["tb"].values()),
                default=1)

    nc = bacc.Bacc("TRN2", debug=False)
    nc.num_devices = CORES

    xTf_in = nc.dram_tensor("xTf", [D, CORES * RPC], _BF16,
                            kind="ExternalInput")
    xTo_in = nc.dram_tensor("xTo", [D, RPC], _BF16, kind="ExternalInput")
    identbf_in = nc.dram_tensor("identbf", [P, P], _BF16,
                                kind="ExternalInput")
    dinvf_in = nc.dram_tensor("dinvf", [P, CORES * TILES], _F32,
                              kind="ExternalInput")
    dinv1_in = nc.dram_tensor("dinv1", [P, TILES], _F32, kind="ExternalInput")
    sqd_in = nc.dram_tensor("sqd", [1, RPC], _BF16, kind="ExternalInput")
    w_in = [nc.dram_tensor(f"W{i + 1}", [D, D], _BF16, kind="ExternalInput")
            for i in range(3)]
    bias_in = [nc.dram_tensor(f"bias{i + 1}", [1, D], _BF16,
                              kind="ExternalInput") for i in range(3)]
    iota_in = nc.dram_tensor("iota", [P, P], _BF16, kind="ExternalInput")
    idx_in = nc.dram_tensor("idx16", [P, tot_idx // 16], _I16,
                            kind="ExternalInput")
    drel_in = nc.dram_tensor("dstrel", [P, tot_cols], _F32,
                             kind="ExternalInput")
    out_t = nc.dram_tensor("out", [RPC, D], _F32, kind="ExternalOutput")

    with tile.TileContext(nc) as tc:
        with tc.tile_pool(name="dram", bufs=1, space="DRAM") as dram, \
             tc.tile_pool(name="const", bufs=1) as cst, \
             tc.tile_pool(name="persist", bufs=1) as per, \
             tc.tile_pool(name="xp", bufs=2) as xp, \
             tc.tile_pool(name="chp", bufs=2) as chp, \
             tc.tile_pool(name="msgp", bufs=cfg.MSGBUFS) as msgp, \
             tc.tile_pool(name="idxp", bufs=2) as idxp, \
             tc.tile_pool(name="qp", bufs=cfg.QBUFS) as qp, \
             tc.tile_pool(name="wk", bufs=cfg.WKBUFS) as wk, \
             tc.tile_pool(name="psa", bufs=cfg.PSABUFS, space="PSUM") as psa, \
             tc.tile_pool(name="psg", bufs=2, space="PSUM") as psg, \
             tc.tile_pool(name="pst", bufs=2, space="PSUM") as pst:

            nc.gpsimd.load_library(library_config.mlp)

            g_fulls = [dram.tile([GR, 2 * D], _BF16,
                                 addr_space=("Local" if i == 0 else "Shared"),
                                 name=f"g_full{i}")
                       for i in range(cfg.LAYERS)]
            g_own = dram.tile([RPC, 2 * D], _BF16, name="g_own")

            iota = cst.tile([P, P], _BF16)
            nc.sync.dma_start(iota[:], iota_in[:])
            ident = cst.tile([P, P], _F32)
            make_identity(nc, ident[:])
            identb = cst.tile([P, P], _BF16)
            nc.sync.dma_start(identb[:], identbf_in[:])
            xTo = cst.tile([D, RPC], _BF16)
            nc.sync.dma_start(xTo[:], xTo_in[:])
            dinvf = cst.tile([P, CORES * TILES], _F32)
            nc.sync.dma_start(dinvf[:], dinvf_in[:])
            dinv1 = cst.tile([P, TILES], _F32)
            nc.sync.dma_start(dinv1[:], dinv1_in[:])
            sqd = cst.tile([1, RPC], _BF16)
            nc.sync.dma_start(sqd[:], sqd_in[:])
            ws, bs = [], []
            for i in range(3):
                w = cst.tile([D, D], _BF16, name=f"w{i}")
                nc.sync.dma_start(w[:], w_in[i][:])
                ws.append(w)
                bt = cst.tile([1, D], _BF16, name=f"b{i}")
                nc.sync.dma_start(bt[:], bias_in[i][:])
                bs.append(bt)

            ht = per.tile([D, RPC], _BF16)          # H.T own (layer input)
            g2ch = per.tile([P, TILES, D], _BF16)   # own dinv*(x@W1) for L0

            for L in range(cfg.LAYERS):
                gf = g_fulls[L]
                # ---------- phase A: table = dinv * (H @ W) ----------
                HC = TILES // 2  # half-chunk tiles
                g2half = []      # per-half chunk tiles for the self term
                if L == 0:
                    # own-slice pass for the self-loop term
                    for t in range(TILES):
                        pg2 = psg.tile([P, D], _F32, tag="pg",
                                       name=f"pg2_{t}")
                        nc.tensor.matmul(
                            pg2[:], lhsT=xTo[:, t * P:(t + 1) * P],
                            rhs=ws[0][:], start=True, stop=True)
                        nc.scalar.activation(
                            g2ch[:, t, :], pg2[:], ACT.Copy,
                            scale=dinv1[:, t:t + 1])
                    # replicated: every core computes the FULL table from x
                    for c in range(CORES):
                        for hh in range(2):
                            t0 = hh * HC
                            xc = xp.tile([D, HC * P], _BF16, tag="xc",
                                         name=f"xc{c}_{hh}")
                            nc.sync.dma_start(
                                xc[:], xTf_in[:, c * RPC + t0 * P:
                                              c * RPC + (t0 + HC) * P])
                            gch = chp.tile([P, HC, 2 * D], _BF16, tag="gch",
                                           name=f"gch0_{c}_{hh}")
                            for i in range(HC):
                                t = t0 + i
                                pg = psg.tile([P, D], _F32, tag="pg",
                                              name=f"pg0_{c}_{t}")
                                nc.tensor.matmul(
                                    pg[:], lhsT=xc[:, i * P:(i + 1) * P],
                                    rhs=ws[0][:], start=True, stop=True)
                                gt = c * TILES + t
                                nc.scalar.activation(
                                    gch[:, i, :D], pg[:], ACT.Copy,
                                    scale=dinvf[:, gt:gt + 1])
                            nc.sync.dma_start(
                                gf[c * RPC:(c + 1) * RPC, :].rearrange(
                                    "(r t) d -> r t d", r=P, t=TILES
                                )[:, t0:t0 + HC, :],
                                gch[:])
                else:
                    for hh in range(2):
                        t0 = hh * HC
                        gch = chp.tile([P, HC, 2 * D], _BF16, tag="gch",
                                       name=f"gch{L}_{hh}")
                        g2half.append(gch)
                        for i in range(HC):
                            t = t0 + i
                            pg = psg.tile([P, D], _F32, tag="pg",
                                          name=f"pg{L}_{t}")
                            nc.tensor.matmul(
                                pg[:], lhsT=ht[:, t * P:(t + 1) * P],
                                rhs=ws[L][:], start=True, stop=True)
                            nc.scalar.activation(
                                gch[:, i, :D], pg[:], ACT.Copy,
                                scale=dinv1[:, t:t + 1])
                        nc.sync.dma_start(
                            g_own[:].rearrange(
                                "(r t) d -> r t d", r=P, t=TILES
                            )[:, t0:t0 + HC, :],
                            gch[:])
                    # ---------- phase B: AllGather ----------
                    if "ag" not in cfg.ABLATE:
                        nc.gpsimd.collective_compute(
                            "AllGather",
                            mybir.AluOpType.bypass,
                            replica_groups=[list(range(CORES))],
                            ins=[g_own[:]],
                            outs=[gf[:]],
                        )

                # ---------- phase C: edge aggregation ----------
                for gi_i, gi in enumerate(ginfos):
                    blocks = gi["blocks"]
                    i0 = gi["blk0"] * P
                    nidx_g = blocks * P
                    msg = msgp.tile([P, gblk_max, 2 * D], _BF16, tag="msg",
                                    name=f"msg{L}_{gi_i}",
                                    padded_shape=[P, gblk_max, 2 * D])
                    idx_sb = idxp.tile([P, (gblk_max * P) // 16], _I16,
                                       tag="idx", name=f"idx{L}_{gi_i}",
                                       padded_shape=[P, (gblk_max * P) // 16])
                    drel_sb = idxp.tile([P, gcol_max], _F32, tag="drel",
                                        name=f"drel{L}_{gi_i}",
                                        padded_shape=[P, gcol_max])
                    nc.sync.dma_start(
                        idx_sb[:, :nidx_g // 16],
                        idx_in[:, i0 // 16:(i0 + nidx_g) // 16],
                    )
                    nc.sync.dma_start(
                        drel_sb[:, :gi["ncols"]],
                        drel_in[:, gi["col0"]:gi["col0"] + gi["ncols"]],
                    )
                    for b, (bo, kgb) in gi["spans"].items():
                        if "gather" in cfg.ABLATE:
                            continue
                        for s0 in range(0, kgb, cfg.GCAP):
                            kk = min(cfg.GCAP, kgb - s0)
                            bo2 = bo + s0
                            nidx = kk * P
                            nc.gpsimd.dma_gather(
                                msg[:, bo2:bo2 + kk, :],
                                gf[b * BUCK_ROWS:(b + 1) * BUCK_ROWS, :],
                                idx_sb[:, bo2 * P // 16:
                                       (bo2 * P + nidx) // 16],
                                nidx, nidx, 2 * D,
                                single_packet=cfg.SPACK,
                            )

                    for t in gi["tiles"]:
                        tbs = [(b, gi["tb"][(t, b)]) for b in range(NBUCK)
                               if (t, b) in gi["tb"]]
                        if "mm" in cfg.ABLATE:
                            tbs = []
                        pa = psa.tile([P, D], _F32, tag="pa",
                                      name=f"pa{L}_{t}")
                        # bias via rank-1 matmul: pa += sqrtdeg^T b
                        nc.tensor.matmul(
                            pa[:], lhsT=sqd[0:1, t * P:(t + 1) * P],
                            rhs=bs[L][:], start=True, stop=False)
                        # self loop: pa += I @ (dinv * own (H@W))[tile t]
                        g2rhs = (g2ch[:, t, :] if L == 0 else
                                 g2half[t // HC][:, t % HC, :D])
                        nc.tensor.matmul(
                            pa[:], lhsT=identb[:], rhs=g2rhs,
                            start=False, stop=(len(tbs) == 0))
                        for bi, (b, (jb, njt, colg, so)) in enumerate(tbs):
                            q = qp.tile([P, njt, P], _BF16, tag="q",
                                        name=f"q{L}_{t}_{b}",
                                        padded_shape=[P, kmaxb, P])
                            if "q" not in cfg.ABLATE:
                                for j in range(njt):
                                    nc.vector.tensor_scalar(
                                        out=q[:, j, :], in0=iota[:],
                                        scalar1=drel_sb[:, colg + j:
                                                        colg + j + 1],
                                        scalar2=None,
                                        op0=mybir.AluOpType.is_equal,
                                    )
                            for j in range(njt):
                                nc.tensor.matmul(
                                    pa[:], lhsT=q[:, j, :],
                                    rhs=msg[:, jb + j, :D],
                                    start=False,
                                    stop=(bi == len(tbs) - 1 and
                                          j == njt - 1),
                                )
                        h = wk.tile([P, D], _F32, tag="h", name=f"h{L}_{t}")
                        nc.scalar.activation(
                            h[:], pa[:], ACT.Lrelu,
                            scale=dinv1[:, t:t + 1], alpha=NEG_SLOPE)
                        if L < cfg.LAYERS - 1:
                            pt = pst.tile([D, P], _F32, tag="pt",
                                          name=f"pt{L}_{t}")
                            nc.tensor.transpose(pt[:], h[:], ident[:])
                            nc.scalar.activation(
                                ht[:, t * P:(t + 1) * P], pt[:], ACT.Copy)
                        else:
                            nc.sync.dma_start(out_t[t * P:(t + 1) * P, :],
                                              h[:])

    nc.compile()
    return nc


def make_in_maps(x, Ws, bss, meta, per_core, cfg):
    dinv = meta["dinv"]
    CORES, NPC, RPC, TILES = cfg.CORES, cfg.NPC, cfg.RPC, cfg.TILES
    import ml_dtypes
    iota_np = np.broadcast_to(np.arange(P).astype(ml_dtypes.bfloat16),
                              (P, P)).copy()

    # full x, transposed, tile-padded per core block: [D, CORES*RPC]
    xpad = np.zeros((CORES * RPC, D), np.float32)
    for c in range(CORES):
        xpad[c * RPC:c * RPC + NPC] = x[c * NPC:(c + 1) * NPC]
    xTf = xpad.T.astype(ml_dtypes.bfloat16).copy()

    # dinv in permuted layout [128, CORES*TILES]
    dpad = np.zeros((CORES * RPC,), np.float32)
    for c in range(CORES):
        dpad[c * RPC:c * RPC + NPC] = dinv[c * NPC:(c + 1) * NPC]
    dinvf = dpad.reshape(CORES * TILES, P).T.copy()  # [r, c*TILES+t]

    in_maps = []
    for c in range(CORES):
        d1 = dinvf[:, c * TILES:(c + 1) * TILES].copy()
        dq = dpad[c * RPC:(c + 1) * RPC]
        sq = np.zeros((1, RPC), np.float32)
        nz = dq > 0
        sq[0, nz] = 1.0 / dq[nz]
        sq = sq.astype(ml_dtypes.bfloat16)
        im = {
            "xTf": xTf,
            "xTo": xTf[:, c * RPC:(c + 1) * RPC].copy(),
            "identbf": np.eye(P, dtype=ml_dtypes.bfloat16),
            "dinvf": dinvf,
            "dinv1": d1,
            "sqd": sq,
            "iota": iota_np,
            "idx16": per_core[c]["idx16"],
            "dstrel": per_core[c]["dstrel"],
        }
        for i in range(3):
            im[f"W{i + 1}"] = Ws[i].astype(ml_dtypes.bfloat16)
            im[f"bias{i + 1}"] = bss[i].reshape(1, D).astype(ml_dtypes.bfloat16)
        in_maps.append(im)
    return in_maps


_CACHE = {}


def kernel(x, edge_index, W1, b1, W2, b2, W3, b3):
    cfg = DEFAULT_CFG
    x = np.asarray(x, dtype=np.float32)
    Ws = [np.asarray(w, dtype=np.float32) for w in (W1, W2, W3)]
    bss = [np.asarray(b, dtype=np.float32) for b in (b1, b2, b3)]

    ei = np.asarray(edge_index)
    key = hash(ei[:, ::997].tobytes()) ^ hash(ei.shape)
    if key not in _CACHE:
        meta, per_core = _preprocess(ei, cfg)
        nc = _build_program(meta, cfg)
        _CACHE[key] = (meta, per_core, nc)
    meta, per_core, nc = _CACHE[key]

    in_maps = make_in_maps(x, Ws, bss, meta, per_core, cfg)
    res = run_bass_kernel_spmd(nc, in_maps, core_ids=list(range(cfg.CORES)))
    out = np.empty((cfg.N, D), np.float32)
    for c in range(cfg.CORES):
        out[c * cfg.NPC:(c + 1) * cfg.NPC] = res.results[c]["out"][:cfg.NPC]
    return out


if __name__ == "__main__":
    # quick smoke: build only
    rng = np.random.default_rng(0)
    ei = rng.integers(0, DEFAULT_CFG.N, size=(2, 1600000))
    meta, per_core = _preprocess(ei, DEFAULT_CFG)
    print("tot_idx", meta["tot_idx"], "tot_cols", meta["tot_cols"])
    nc = _build_program(meta, DEFAULT_CFG)
    print("build ok")


# revision 26
# speedup vs baseline: 1.0704x; 1.0115x over previous
"""3-layer GCN (GCNConv x3 + LeakyReLU, PyG semantics) on 8 Trainium2 cores.

Strategy (graph-parallel over destination nodes):
  - Nodes are partitioned into 8 contiguous ranges; core c owns range c and
    computes the output rows for its own nodes.
  - Table rows are laid out PERMUTED: row'(v) = c*RPC + r*TILES + tc where
    v = c*NPC + tc*128 + r.  A phase-A chunk [128, TILES, 2D] then writes to
    DRAM with per-partition-contiguous runs (fast big-elem DMA), and bucket
    b = rows [b*25088, (b+1)*25088) corresponds to cores {2b, 2b+1}.
  - Layer 1: x is a full input, so EVERY core computes the full table
    locally (replicated phase A from xT in bf16) -- no AllGather at all.
  - Layers 2-3: phase A computes the own chunk; AllGather builds the full
    table.
  - Self-loops are NOT gathered: the self term is added with an identity
    matmul against the own phase-A chunk; the bias is injected via a
    rank-1 matmul (sqrt(deg) outer b) into the same PSUM accumulator, so the
    epilogue is a single Activation-engine LeakyRelu with per-partition
    dinv scale:  h = lrelu(dinv * (sum_edges table[src] + self + sqrtdeg*b)).
  - dma_gather indices are int16; the table (100352 rows) is addressed
    through 4 bucket views of 25088 rows; edges are bucketed by row' range.

The Bass program is SPMD: one program, per-core input data. Per
(tile, bucket) section lengths are shared across cores (max over cores,
padded with dummy edges whose one-hot column is zero: dstrel = -1).
"""
import sys

sys.path.insert(0, "/opt/trn_rl_repo")

import numpy as np

import concourse.bacc as bacc
import concourse.mybir as mybir
import concourse.tile as tile
from concourse import library_config
from concourse.bass_utils import run_bass_kernel_spmd
from concourse.masks import make_identity

_F32 = mybir.dt.float32
_BF16 = mybir.dt.bfloat16
_I16 = mybir.dt.int16
P = 128
D = 64
NEG_SLOPE = 0.01
ACT = mybir.ActivationFunctionType


class Cfg:
    def __init__(self, n_nodes=100000, cores=8, group=4, layers=3):
        self.N = n_nodes
        self.CORES = cores
        self.NPC = self.N // cores            # nodes owned per core
        self.TILES = (self.NPC + P - 1) // P  # dst tiles per core
        self.RPC = self.TILES * P             # padded rows per core
        self.GR = cores * self.RPC            # gather-table rows
        self.NBUCK = max(1, -(-self.GR // 25088))
        assert self.GR % self.NBUCK == 0
        self.BUCK_ROWS = self.GR // self.NBUCK
        assert self.BUCK_ROWS <= 32767
        self.GROUP = group                    # dst tiles per gather group
        self.LAYERS = layers
        self.ABLATE = set()
        self.GCAP = 32                        # max 128-blocks per dma_gather
        self.SPACK = False
        self.MSGBUFS = 2
        self.QBUFS = 6
        self.PSABUFS = 4
        self.WKBUFS = 3


DEFAULT_CFG = Cfg()


def _preprocess(edge_index, cfg):
    """Sort/bucket/pack edges; build per-core device arrays and metadata."""
    N, CORES, NPC, RPC = cfg.N, cfg.CORES, cfg.NPC, cfg.RPC
    TILES, NBUCK, BUCK_ROWS = cfg.TILES, cfg.NBUCK, cfg.BUCK_ROWS

    rsrc = np.asarray(edge_index[0], dtype=np.int64)
    rdst = np.asarray(edge_index[1], dtype=np.int64)

    deg = np.bincount(rdst, minlength=N).astype(np.float32) + 1.0  # + self loop
    dinv = (1.0 / np.sqrt(deg)).astype(np.float32)

    # self loops handled by an identity matmul against the phase-A chunk
    src = rsrc
    dst = rdst

    c_s = src // NPC
    loc_s = src - c_s * NPC
    rowp = c_s * RPC + (loc_s % P) * TILES + (loc_s // P)  # permuted table row
    bucket = rowp // BUCK_ROWS
    lidx = rowp % BUCK_ROWS

    owner = dst // NPC
    dloc = dst - owner * NPC
    tile_id = dloc // P
    dstrel = dloc % P

    counts = np.zeros((CORES, TILES, NBUCK), dtype=np.int64)
    np.add.at(counts, (owner, tile_id, bucket), 1)
    order = np.lexsort((bucket, tile_id, owner))
    sl = lidx[order]
    sr = dstrel[order]

    sec_len = counts.max(axis=0)              # [TILES, NBUCK] shared sections
    groups = [list(range(g, min(g + cfg.GROUP, TILES)))
              for g in range(0, TILES, cfg.GROUP)]

    # layout: group -> bucket -> tile sections back-to-back, span padded to
    # a multiple of 128 at the end
    ginfos = []
    tot_blocks = 0
    tot_cols = 0
    for grp in groups:
        gi = {"tiles": list(grp), "blk0": tot_blocks, "col0": tot_cols,
              "spans": {}, "tb": {}}
        gblk = 0
        gcol = 0
        for b in range(NBUCK):
            span_len = int(sec_len[grp, b].sum())
            if span_len == 0:
                continue
            kgb = -(-span_len // P)
            gi["spans"][b] = (gblk, kgb)
            so = 0
            for t in grp:
                stb = int(sec_len[t, b])
                if stb == 0:
                    continue
                j0 = so // P
                njt = (so + stb - 1) // P - j0 + 1
                gi["tb"][(t, b)] = (gblk + j0, njt, gcol, so)
                gcol += njt
                so += stb
            gblk += kgb
        gi["blocks"] = gblk
        gi["ncols"] = gcol
        tot_blocks += gblk
        tot_cols += gcol
        ginfos.append(gi)
    tot_idx = tot_blocks * P

    # boundaries of each core's (t, b) run in the sorted edge list
    cum = np.zeros((CORES, TILES, NBUCK + 1), dtype=np.int64)
    cum[:, :, 1:] = np.cumsum(counts, axis=2)
    flat_counts = counts.sum(axis=2)
    run = np.cumsum(flat_counts.reshape(-1))
    base = np.zeros(CORES * TILES, dtype=np.int64)
    base[1:] = run[:-1]
    base = base.reshape(CORES, TILES)

    per_core = []
    for c in range(CORES):
        lidx_flat = np.zeros(tot_idx, dtype=np.int16)
        drel = np.full((P, tot_cols), -1.0, dtype=np.float32)
        for gi in ginfos:
            for b, (bo, kgb) in gi["spans"].items():
                span_i0 = (gi["blk0"] + bo) * P
                for t in gi["tiles"]:
                    if (t, b) not in gi["tb"]:
                        continue
                    jb, njt, colg, so = gi["tb"][(t, b)]
                    n = int(counts[c, t, b])
                    if n == 0:
                        continue
                    s0 = int(base[c, t] + cum[c, t, b])
                    pos0 = span_i0 + so
                    lidx_flat[pos0:pos0 + n] = sl[s0:s0 + n].astype(np.int16)
                    q = so + np.arange(n)
                    jrel = q // P - so // P
                    pp = (pos0 + np.arange(n)) % P
                    cols = gi["col0"] + colg + jrel
                    drel[pp, cols] = sr[s0:s0 + n].astype(np.float32)
        idx16 = np.tile(lidx_flat.reshape(tot_idx // 16, 16).T, (8, 1)).copy()
        per_core.append({"idx16": idx16, "dstrel": drel})

    meta = {
        "sec_len": sec_len,
        "groups": groups,
        "ginfos": ginfos,
        "tot_idx": tot_idx,
        "tot_cols": tot_cols,
        "dinv": dinv,
    }
    return meta, per_core


def _build_program(meta, cfg):
    ginfos = meta["ginfos"]
    tot_idx = meta["tot_idx"]
    tot_cols = meta["tot_cols"]
    CORES, TILES, RPC, GR = cfg.CORES, cfg.TILES, cfg.RPC, cfg.GR
    NBUCK, BUCK_ROWS = cfg.NBUCK, cfg.BUCK_ROWS

    gblk_max = max(gi["blocks"] for gi in ginfos)
    gcol_max = max(gi["ncols"] for gi in ginfos)
    kmaxb = max((tb[1] for gi in ginfos for tb in gi["tb"].values()),
                default=1)

    nc = bacc.Bacc("TRN2", debug=False)
    nc.num_devices = CORES

    xTf_in = nc.dram_tensor("xTf", [D, CORES * RPC], _BF16,
                            kind="ExternalInput")
    xTo_in = nc.dram_tensor("xTo", [D, RPC], _BF16, kind="ExternalInput")
    identbf_in = nc.dram_tensor("identbf", [P, P], _BF16,
                                kind="ExternalInput")
    dinvf_in = nc.dram_tensor("dinvf", [P, CORES * TILES], _F32,
                              kind="ExternalInput")
    dinv1_in = nc.dram_tensor("dinv1", [P, TILES], _F32, kind="ExternalInput")
    sqd_in = nc.dram_tensor("sqd", [1, RPC], _BF16, kind="ExternalInput")
    w_in = [nc.dram_tensor(f"W{i + 1}", [D, D], _BF16, kind="ExternalInput")
            for i in range(3)]
    bias_in = [nc.dram_tensor(f"bias{i + 1}", [1, D], _BF16,
                              kind="ExternalInput") for i in range(3)]
    iota_in = nc.dram_tensor("iota", [P, P], _BF16, kind="ExternalInput")
    idx_in = nc.dram_tensor("idx16", [P, tot_idx // 16], _I16,
                            kind="ExternalInput")
    drel_in = nc.dram_tensor("dstrel", [P, tot_cols], _F32,
                             kind="ExternalInput")
    out_t = nc.dram_tensor("out", [RPC, D], _F32, kind="ExternalOutput")

    with tile.TileContext(nc) as tc:
        with tc.tile_pool(name="dram", bufs=1, space="DRAM") as dram, \
             tc.tile_pool(name="const", bufs=1) as cst, \
             tc.tile_pool(name="persist", bufs=1) as per, \
             tc.tile_pool(name="xp", bufs=2) as xp, \
             tc.tile_pool(name="chp", bufs=2) as chp, \
             tc.tile_pool(name="msgp", bufs=cfg.MSGBUFS) as msgp, \
             tc.tile_pool(name="idxp", bufs=2) as idxp, \
             tc.tile_pool(name="qp", bufs=cfg.QBUFS) as qp, \
             tc.tile_pool(name="wk", bufs=cfg.WKBUFS) as wk, \
             tc.tile_pool(name="psa", bufs=cfg.PSABUFS, space="PSUM") as psa, \
             tc.tile_pool(name="psg", bufs=2, space="PSUM") as psg, \
             tc.tile_pool(name="pst", bufs=2, space="PSUM") as pst:

            nc.gpsimd.load_library(library_config.mlp)

            g_fulls = [dram.tile([GR, 2 * D], _BF16,
                                 addr_space=("Local" if i == 0 else "Shared"),
                                 name=f"g_full{i}")
                       for i in range(cfg.LAYERS)]
            g_own = dram.tile([RPC, 2 * D], _BF16, name="g_own")

            iota = cst.tile([P, P], _BF16)
            nc.sync.dma_start(iota[:], iota_in[:])
            ident = cst.tile([P, P], _F32)
            make_identity(nc, ident[:])
            identb = cst.tile([P, P], _BF16)
            nc.sync.dma_start(identb[:], identbf_in[:])
            xTo = cst.tile([D, RPC], _BF16)
            nc.sync.dma_start(xTo[:], xTo_in[:])
            dinvf = cst.tile([P, CORES * TILES], _F32)
            nc.sync.dma_start(dinvf[:], dinvf_in[:])
            dinv1 = cst.tile([P, TILES], _F32)
            nc.sync.dma_start(dinv1[:], dinv1_in[:])
            sqd = cst.tile([1, RPC], _BF16)
            nc.sync.dma_start(sqd[:], sqd_in[:])
            ws, bs = [], []
            for i in range(3):
                w = cst.tile([D, D], _BF16, name=f"w{i}")
                nc.sync.dma_start(w[:], w_in[i][:])
                ws.append(w)
                bt = cst.tile([1, D], _BF16, name=f"b{i}")
                nc.sync.dma_start(bt[:], bias_in[i][:])
                bs.append(bt)

            ht = per.tile([D, RPC], _BF16)          # H.T own (layer input)
            g2ch = per.tile([P, TILES, D], _BF16)   # own dinv*(x@W1) for L0

            for L in range(cfg.LAYERS):
                gf = g_fulls[L]
                # ---------- phase A: table = dinv * (H @ W) ----------
                HC = TILES // 2  # half-chunk tiles
                g2half = []      # per-half chunk tiles for the self term
                if L == 0:
                    # own-slice pass for the self-loop term
                    for t in range(TILES):
                        pg2 = psg.tile([P, D], _F32, tag="pg",
                                       name=f"pg2_{t}")
                        nc.tensor.matmul(
                            pg2[:], lhsT=xTo[:, t * P:(t + 1) * P],
                            rhs=ws[0][:], start=True, stop=True)
                        nc.scalar.activation(
                            g2ch[:, t, :], pg2[:], ACT.Copy,
                            scale=dinv1[:, t:t + 1])
                    # replicated: every core computes the FULL table from x
                    for c in range(CORES):
                        for hh in range(2):
                            t0 = hh * HC
                            xc = xp.tile([D, HC * P], _BF16, tag="xc",
                                         name=f"xc{c}_{hh}")
                            nc.sync.dma_start(
                                xc[:], xTf_in[:, c * RPC + t0 * P:
                                              c * RPC + (t0 + HC) * P])
                            gch = chp.tile([P, HC, 2 * D], _BF16, tag="gch",
                                           name=f"gch0_{c}_{hh}")
                            for i in range(HC):
                                t = t0 + i
                                pg = psg.tile([P, D], _F32, tag="pg",
                                              name=f"pg0_{c}_{t}")
                                nc.tensor.matmul(
                                    pg[:], lhsT=xc[:, i * P:(i + 1) * P],
                                    rhs=ws[0][:], start=True, stop=True)
                                gt = c * TILES + t
                                # alternate engines: Act's serial chain was
                                # the lead-in bottleneck of the replicated
                                # pass
                                if i % 2 == 0:
                                    nc.scalar.activation(
                                        gch[:, i, :D], pg[:], ACT.Copy,
                                        scale=dinvf[:, gt:gt + 1])
                                else:
                                    nc.vector.tensor_scalar_mul(
                                        gch[:, i, :D], pg[:],
                                        dinvf[:, gt:gt + 1])
                            nc.sync.dma_start(
                                gf[c * RPC:(c + 1) * RPC, :].rearrange(
                                    "(r t) d -> r t d", r=P, t=TILES
                                )[:, t0:t0 + HC, :],
                                gch[:])
                else:
                    for hh in range(2):
                        t0 = hh * HC
                        gch = chp.tile([P, HC, 2 * D], _BF16, tag="gch",
                                       name=f"gch{L}_{hh}")
                        g2half.append(gch)
                        for i in range(HC):
                            t = t0 + i
                            pg = psg.tile([P, D], _F32, tag="pg",
                                          name=f"pg{L}_{t}")
                            nc.tensor.matmul(
                                pg[:], lhsT=ht[:, t * P:(t + 1) * P],
                                rhs=ws[L][:], start=True, stop=True)
                            nc.scalar.activation(
                                gch[:, i, :D], pg[:], ACT.Copy,
                                scale=dinv1[:, t:t + 1])
                        nc.sync.dma_start(
                            g_own[:].rearrange(
                                "(r t) d -> r t d", r=P, t=TILES
                            )[:, t0:t0 + HC, :],
                            gch[:])
                    # ---------- phase B: AllGather ----------
                    if "ag" not in cfg.ABLATE:
                        nc.gpsimd.collective_compute(
                            "AllGather",
                            mybir.AluOpType.bypass,
                            replica_groups=[list(range(CORES))],
                            ins=[g_own[:]],
                            outs=[gf[:]],
                        )

                # ---------- phase C: edge aggregation ----------
                for gi_i, gi in enumerate(ginfos):
                    blocks = gi["blocks"]
                    i0 = gi["blk0"] * P
                    nidx_g = blocks * P
                    msg = msgp.tile([P, gblk_max, 2 * D], _BF16, tag="msg",
                                    name=f"msg{L}_{gi_i}",
                                    padded_shape=[P, gblk_max, 2 * D])
                    idx_sb = idxp.tile([P, (gblk_max * P) // 16], _I16,
                                       tag="idx", name=f"idx{L}_{gi_i}",
                                       padded_shape=[P, (gblk_max * P) // 16])
                    drel_sb = idxp.tile([P, gcol_max], _F32, tag="drel",
                                        name=f"drel{L}_{gi_i}",
                                        padded_shape=[P, gcol_max])
                    nc.sync.dma_start(
                        idx_sb[:, :nidx_g // 16],
                        idx_in[:, i0 // 16:(i0 + nidx_g) // 16],
                    )
                    nc.sync.dma_start(
                        drel_sb[:, :gi["ncols"]],
                        drel_in[:, gi["col0"]:gi["col0"] + gi["ncols"]],
                    )
                    for b, (bo, kgb) in gi["spans"].items():
                        if "gather" in cfg.ABLATE:
                            continue
                        for s0 in range(0, kgb, cfg.GCAP):
                            kk = min(cfg.GCAP, kgb - s0)
                            bo2 = bo + s0
                            nidx = kk * P
                            nc.gpsimd.dma_gather(
                                msg[:, bo2:bo2 + kk, :],
                                gf[b * BUCK_ROWS:(b + 1) * BUCK_ROWS, :],
                                idx_sb[:, bo2 * P // 16:
                                       (bo2 * P + nidx) // 16],
                                nidx, nidx, 2 * D,
                                single_packet=cfg.SPACK,
                            )

                    for t in gi["tiles"]:
                        tbs = [(b, gi["tb"][(t, b)]) for b in range(NBUCK)
                               if (t, b) in gi["tb"]]
                        if "mm" in cfg.ABLATE:
                            tbs = []
                        pa = psa.tile([P, D], _F32, tag="pa",
                                      name=f"pa{L}_{t}")
                        # bias via rank-1 matmul: pa += sqrtdeg^T b
                        nc.tensor.matmul(
                            pa[:], lhsT=sqd[0:1, t * P:(t + 1) * P],
                            rhs=bs[L][:], start=True, stop=False)
                        # self loop: pa += I @ (dinv * own (H@W))[tile t]
                        g2rhs = (g2ch[:, t, :] if L == 0 else
                                 g2half[t // HC][:, t % HC, :D])
                        nc.tensor.matmul(
                            pa[:], lhsT=identb[:], rhs=g2rhs,
                            start=False, stop=(len(tbs) == 0))
                        for bi, (b, (jb, njt, colg, so)) in enumerate(tbs):
                            q = qp.tile([P, njt, P], _BF16, tag="q",
                                        name=f"q{L}_{t}_{b}",
                                        padded_shape=[P, kmaxb, P])
                            if "q" not in cfg.ABLATE:
                                for j in range(njt):
                                    nc.vector.tensor_scalar(
                                        out=q[:, j, :], in0=iota[:],
                                        scalar1=drel_sb[:, colg + j:
                                                        colg + j + 1],
                                        scalar2=None,
                                        op0=mybir.AluOpType.is_equal,
                                    )
                            for j in range(njt):
                                nc.tensor.matmul(
                                    pa[:], lhsT=q[:, j, :],
                                    rhs=msg[:, jb + j, :D],
                                    start=False,
                                    stop=(bi == len(tbs) - 1 and
                                          j == njt - 1),
                                )
                        h = wk.tile([P, D], _F32, tag="h", name=f"h{L}_{t}")
                        nc.scalar.activation(
                            h[:], pa[:], ACT.Lrelu,
                            scale=dinv1[:, t:t + 1], alpha=NEG_SLOPE)
                        if L < cfg.LAYERS - 1:
                            pt = pst.tile([D, P], _F32, tag="pt",
                                          name=f"pt{L}_{t}")
                            nc.tensor.transpose(pt[:], h[:], ident[:])
                            nc.scalar.activation(
                                ht[:, t * P:(t + 1) * P], pt[:], ACT.Copy)
                        else:
                            nc.sync.dma_start(out_t[t * P:(t + 1) * P, :],
                                              h[:])

    nc.compile()
    return nc


def make_in_maps(x, Ws, bss, meta, per_core, cfg):
    dinv = meta["dinv"]
    CORES, NPC, RPC, TILES = cfg.CORES, cfg.NPC, cfg.RPC, cfg.TILES
    import ml_dtypes
    iota_np = np.broadcast_to(np.arange(P).astype(ml_dtypes.bfloat16),
                              (P, P)).copy()

    # full x, transposed, tile-padded per core block: [D, CORES*RPC]
    xpad = np.zeros((CORES * RPC, D), np.float32)
    for c in range(CORES):
        xpad[c * RPC:c * RPC + NPC] = x[c * NPC:(c + 1) * NPC]
    xTf = xpad.T.astype(ml_dtypes.bfloat16).copy()

    # dinv in permuted layout [128, CORES*TILES]
    dpad = np.zeros((CORES * RPC,), np.float32)
    for c in range(CORES):
        dpad[c * RPC:c * RPC + NPC] = dinv[c * NPC:(c + 1) * NPC]
    dinvf = dpad.reshape(CORES * TILES, P).T.copy()  # [r, c*TILES+t]

    in_maps = []
    for c in range(CORES):
        d1 = dinvf[:, c * TILES:(c + 1) * TILES].copy()
        dq = dpad[c * RPC:(c + 1) * RPC]
        sq = np.zeros((1, RPC), np.float32)
        nz = dq > 0
        sq[0, nz] = 1.0 / dq[nz]
        sq = sq.astype(ml_dtypes.bfloat16)
        im = {
            "xTf": xTf,
            "xTo": xTf[:, c * RPC:(c + 1) * RPC].copy(),
            "identbf": np.eye(P, dtype=ml_dtypes.bfloat16),
            "dinvf": dinvf,
            "dinv1": d1,
            "sqd": sq,
            "iota": iota_np,
            "idx16": per_core[c]["idx16"],
            "dstrel": per_core[c]["dstrel"],
        }
        for i in range(3):
            im[f"W{i + 1}"] = Ws[i].astype(ml_dtypes.bfloat16)
            im[f"bias{i + 1}"] = bss[i].reshape(1, D).astype(ml_dtypes.bfloat16)
        in_maps.append(im)
    return in_maps


_CACHE = {}


def kernel(x, edge_index, W1, b1, W2, b2, W3, b3):
    cfg = DEFAULT_CFG
    x = np.asarray(x, dtype=np.float32)
    Ws = [np.asarray(w, dtype=np.float32) for w in (W1, W2, W3)]
    bss = [np.asarray(b, dtype=np.float32) for b in (b1, b2, b3)]

    ei = np.asarray(edge_index)
    key = hash(ei[:, ::997].tobytes()) ^ hash(ei.shape)
    if key not in _CACHE:
        meta, per_core = _preprocess(ei, cfg)
        nc = _build_program(meta, cfg)
        _CACHE[key] = (meta, per_core, nc)
    meta, per_core, nc = _CACHE[key]

    in_maps = make_in_maps(x, Ws, bss, meta, per_core, cfg)
    res = run_bass_kernel_spmd(nc, in_maps, core_ids=list(range(cfg.CORES)))
    out = np.empty((cfg.N, D), np.float32)
    for c in range(cfg.CORES):
        out[c * cfg.NPC:(c + 1) * cfg.NPC] = res.results[c]["out"][:cfg.NPC]
    return out


if __name__ == "__main__":
    # quick smoke: build only
    rng = np.random.default_rng(0)
    ei = rng.integers(0, DEFAULT_CFG.N, size=(2, 1600000))
    meta, per_core = _preprocess(ei, DEFAULT_CFG)
    print("tot_idx", meta["tot_idx"], "tot_cols", meta["tot_cols"])
    nc = _build_program(meta, DEFAULT_CFG)
    print("build ok")
